# revision 2
# baseline (speedup 1.0000x reference)
"""Trainium2 Bass kernel for nn_BlockDiagonalLinear_text (hyperbolic block-diag linear).

Math: the reference's per-row operations are all scalar row-scalings, so
  out = alpha_row * y   with  y = x @ blockdiag(W_1..W_16).T
where alpha_row is a chain of tanh/artanh/sqrt scalars of ||x_row|| and
||y_row||.  (The expmap scale s cancels out of res_c except through
saturated tanh arguments - validated numerically against the reference.)

Sharding: data-parallel over rows. 8192 rows -> 8 cores x 1024 rows.
Weights (4 MB + identity) replicated. Per-core kernel streams 8 tiles of
128 rows:
  DMA x tile -> ACT x^2 row-sums -> PE transpose x (128x128 blocks) ->
  fp32r (FP22 single-pass) block matmuls -> DVE copy y to SBUF ->
  ACT y^2 row-sums -> per-row scalar chain ([128,1] ops) ->
  DVE scale y in place -> DMA out.

Uses bacc.Bacc (not raw bass.Bass): its compile() pass legalizes
semaphore waits for the 1-wait-per-instruction TPB ISA (EVSEM splitting,
matmul-wait relocation to LDWEIGHTS).
"""
import sys
import numpy as np

for _p in ("/opt/trn_rl_repo", "/root/.axon_site/_ro/trn_rl_repo"):
    if _p not in sys.path:
        sys.path.append(_p)

import concourse.bass as bass
import concourse.bacc as bacc
import concourse.mybir as mybir
from concourse import tile
from concourse.bass_utils import run_bass_kernel_spmd

R, BS = 16, 256           # 16 diagonal blocks of 256x256
D = R * BS                # 4096
P = 128                   # partitions
N_CORES = 8
ROWS_TOTAL = 4 * 2048     # 8192
ROWS_CORE = ROWS_TOTAL // N_CORES   # 1024
NT = ROWS_CORE // P       # 8 tiles of 128 rows per core
WCOLS = 2 * R * BS        # 8192 weight columns
WIDC = WCOLS + P          # + identity columns

f32 = mybir.dt.float32
f32r = mybir.dt.float32r
AF = mybir.ActivationFunctionType
OP = mybir.AluOpType

CLIP_Z = float(np.float32(1.0) - np.float32(1e-5))          # 0.99999
MAXNORM = float(np.float32(1.0 - 1e-3) / np.float32(0.1))   # 9.99


def build_nc(ablate=()):
    ablate = set(ablate)
    nc = bacc.Bacc()
    # float32r (FP22) end-to-end on the matmul path: walrus requires every
    # producer feeding an fp32r matmul to declare an fp32r output.
    x_d = nc.declare_dram_parameter("x", [ROWS_CORE, D], f32r, isOutput=False)
    w_d = nc.declare_dram_parameter("w", [P, WIDC], f32r, isOutput=False)
    out_d = nc.declare_dram_parameter("out", [ROWS_CORE, D], f32, isOutput=True)

    with tile.TileContext(nc) as tc:
        with (
            tc.tile_pool(name="wpool", bufs=1) as wpool,
            tc.tile_pool(name="xpool", bufs=2) as xpool,
            tc.tile_pool(name="ypool", bufs=3) as ypool,
            tc.tile_pool(name="xtpool", bufs=2) as xtpool,
            tc.tile_pool(name="scrpool", bufs=1) as scrpool,
            tc.tile_pool(name="stats", bufs=2) as stats,
            tc.tile_pool(name="pst", bufs=2, space="PSUM") as pst,
            tc.tile_pool(name="psy", bufs=4, space="PSUM") as psy,
        ):
            w_sb = wpool.tile([P, WIDC], f32r, name="w_sb")
            nc.sync.dma_start(out=w_sb[:], in_=w_d[:])
            id_sb = w_sb[:, WCOLS:WIDC]
            scratch = scrpool.tile([P, D], f32, name="scratch")

            def st(shape, tag):
                return stats.tile(shape, f32, tag=tag, name=tag)

            for i in range(NT):
                x_sb = xpool.tile([P, D], f32r, tag="x", name=f"x_{i}")
                nc.sync.dma_start(out=x_sb[:], in_=x_d[i * P:(i + 1) * P, :])

                q2 = st([P, 2], "q2")
                # qx = sum_k x^2 (row-wise)
                if "squares" not in ablate:
                    nc.scalar.activation(scratch[:], x_sb[:].bitcast(f32),
                                         AF.Square, accum_out=q2[:, 0:1])

                # transpose x tile: xt[:, c*128:+128] = x[:, c*128:+128].T
                xt_sb = xtpool.tile([P, D], f32r, tag="xt", name=f"xt_{i}")
                for c in range(D // P):
                    tp = pst.tile([P, P], f32r, tag="tp", name=f"tp_{i}_{c}")
                    nc.tensor.transpose(tp[:], x_sb[:, c * P:(c + 1) * P], id_sb)
                    nc.vector.tensor_copy(xt_sb[:, c * P:(c + 1) * P], tp[:])

                # block matmuls: y[:, r*256:+256] = x_blk_r @ W_r.T  (fp32r)
                y_sb = ypool.tile([P, D], f32, tag="y", name=f"y_{i}")
                for r in range(R):
                    py = psy.tile([P, BS], f32, tag="py", name=f"py_{i}_{r}")
                    for c in range(2):
                        kc = 2 * r + c
                        nc.tensor.matmul(
                            py[:],
                            xt_sb[:, kc * P:(kc + 1) * P],
                            w_sb[:, kc * BS:(kc + 1) * BS],
                            start=(c == 0), stop=(c == 1),
                        )
                    nc.vector.tensor_copy(y_sb[:, r * BS:(r + 1) * BS], py[:])

                # qy = sum_j y^2 (row-wise)
                if "squares" not in ablate:
                    nc.scalar.activation(scratch[:], y_sb[:], AF.Square,
                                         accum_out=q2[:, 1:2])

                # ---- per-row scalar chain ([128,1] / [128,2] ops) ----
                V = nc.vector
                if "chain" in ablate:
                    alm = st([P, 1], "alm")
                    V.tensor_scalar_mul(alm[:], q2[:, 1:2], 1.0)
                    if "scale" not in ablate:
                        V.tensor_scalar(out=y_sb[:], in0=y_sb[:], scalar1=alm[:],
                                        scalar2=5.0, op0=OP.mult, op1=OP.mult)
                    nc.sync.dma_start(out=out_d[i * P:(i + 1) * P, :], in_=y_sb[:])
                    continue
                lnq = st([P, 2], "lnq")
                nc.scalar.activation(lnq[:], q2[:], AF.Ln)
                U = st([P, 2], "U")   # [u | y_n] = sqrt via exp(0.5 ln q)
                nc.scalar.activation(U[:], lnq[:], AF.Exp, scale=0.5)

                uc = st([P, 1], "uc")
                V.tensor_scalar_max(uc[:], U[:, 0:1], 1e-5)
                t1 = st([P, 1], "t1")
                V.tensor_scalar_mul(t1[:], uc[:], 0.1)
                r1 = st([P, 1], "r1")
                V.reciprocal(r1[:], t1[:])
                args_ = st([P, 1], "args_")
                V.tensor_scalar_min(args_[:], t1[:], 15.0)
                Es = st([P, 1], "Es")
                nc.scalar.activation(Es[:], args_[:], AF.Exp, scale=2.0)
                e1 = st([P, 1], "e1")
                V.tensor_scalar_add(e1[:], Es[:], 1.0)
                r2 = st([P, 1], "r2")
                V.reciprocal(r2[:], e1[:])
                tsx = st([P, 1], "tsx")   # tanh(0.1 u_c)
                V.tensor_scalar(out=tsx[:], in0=r2[:], scalar1=-2.0, scalar2=1.0,
                                op0=OP.mult, op1=OP.add)
                za = st([P, 1], "za")
                V.tensor_scalar_min(za[:], tsx[:], CLIP_Z)
                L = st([P, 2], "L")
                V.tensor_scalar_add(L[:, 0:1], za[:], 1.0)
                V.tensor_scalar(out=L[:, 1:2], in0=za[:], scalar1=-1.0, scalar2=1.0,
                                op0=OP.mult, op1=OP.add)
                lnL = st([P, 2], "lnL")
                nc.scalar.activation(lnL[:], L[:], AF.Ln)
                d_ = st([P, 1], "d_")     # 2*artanh(za)
                V.tensor_sub(d_[:], lnL[:, 0:1], lnL[:, 1:2])
                yns = st([P, 1], "yns")   # y_n clamped for safe reciprocal
                V.tensor_scalar_max(yns[:], U[:, 1:2], 1e-20)
                w1 = st([P, 1], "w1")
                V.tensor_mul(w1[:], U[:, 1:2], r1[:])
                w2 = st([P, 1], "w2")
                V.tensor_mul(w2[:], w1[:], d_[:])
                argt = st([P, 1], "argt")
                V.tensor_scalar(out=argt[:], in0=w2[:], scalar1=0.05, scalar2=15.0,
                                op0=OP.mult, op1=OP.min)
                Et = st([P, 1], "Et")
                nc.scalar.activation(Et[:], argt[:], AF.Exp, scale=2.0)
                e2 = st([P, 1], "e2")
                V.tensor_scalar_add(e2[:], Et[:], 1.0)
                r3 = st([P, 1], "r3")
                V.reciprocal(r3[:], e2[:])
                ttx = st([P, 1], "ttx")   # tanh(arg_t)
                V.tensor_scalar(out=ttx[:], in0=r3[:], scalar1=-2.0, scalar2=1.0,
                                op0=OP.mult, op1=OP.add)
                nrm = st([P, 1], "nrm")
                V.tensor_scalar(out=nrm[:], in0=ttx[:], scalar1=10.0, scalar2=1e-5,
                                op0=OP.mult, op1=OP.max)
                ryn = st([P, 1], "ryn")
                V.reciprocal(ryn[:], yns[:])
                gs = st([P, 1], "gs")
                V.tensor_mul(gs[:], ttx[:], ryn[:])
                rn = st([P, 1], "rn")
                V.reciprocal(rn[:], nrm[:])
                p9 = st([P, 1], "p9")
                V.tensor_scalar_mul(p9[:], rn[:], MAXNORM)
                pf = st([P, 1], "pf")
                V.tensor_scalar_min(pf[:], p9[:], 1.0)
                m_ = st([P, 1], "m_")
                V.tensor_scalar_min(m_[:], nrm[:], MAXNORM)
                zb = st([P, 1], "zb")
                V.tensor_scalar_mul(zb[:], m_[:], 0.1)
                B = st([P, 2], "B")
                V.tensor_scalar_add(B[:, 0:1], zb[:], 1.0)
                V.tensor_scalar(out=B[:, 1:2], in0=zb[:], scalar1=-1.0, scalar2=1.0,
                                op0=OP.mult, op1=OP.add)
                lnB = st([P, 2], "lnB")
                nc.scalar.activation(lnB[:], B[:], AF.Ln)
                db = st([P, 1], "db")     # 2*artanh(0.1 m)
                V.tensor_sub(db[:], lnB[:, 0:1], lnB[:, 1:2])
                rzb = st([P, 1], "rzb")
                V.reciprocal(rzb[:], zb[:])
                a1 = st([P, 1], "a1")
                V.tensor_mul(a1[:], gs[:], pf[:])
                a2 = st([P, 1], "a2")
                V.tensor_mul(a2[:], db[:], rzb[:])
                al = st([P, 1], "al")
                V.tensor_mul(al[:], a1[:], a2[:])
                mask = st([P, 1], "mask")
                V.tensor_scalar(out=mask[:], in0=q2[:, 1:2], scalar1=0.0, scalar2=None,
                                op0=OP.is_gt)
                alm = st([P, 1], "alm")
                V.tensor_mul(alm[:], al[:], mask[:])

                # out = y * alpha * 5  (5 = 10 from gs x 0.5 from artanh halves)
                if "scale" not in ablate:
                    V.tensor_scalar(out=y_sb[:], in0=y_sb[:], scalar1=alm[:],
                                    scalar2=5.0, op0=OP.mult, op1=OP.mult)
                nc.sync.dma_start(out=out_d[i * P:(i + 1) * P, :], in_=y_sb[:])
    nc.finalize()   # Bacc.compile(): reg alloc + EVSEM wait legalization
    return nc


_NC = None


def _get_nc():
    global _NC
    if _NC is None:
        _NC = build_nc()
    return _NC


def _round_fp22(a: np.ndarray) -> np.ndarray:
    # round-to-nearest-even to 13-bit mantissa (float32r / FP22)
    u = a.astype(np.float32).view(np.uint32)
    keep = np.uint32(0xFFFFFC00)
    low = u & np.uint32(0x3FF)
    half = np.uint32(0x200)
    lsb = (u >> np.uint32(10)) & np.uint32(1)
    round_up = (low > half) | ((low == half) & (lsb == 1))
    u = (u & keep) + (round_up.astype(np.uint32) << np.uint32(10))
    return u.view(np.float32)


def _prep_weights(weights: np.ndarray) -> np.ndarray:
    # w_sb[:, (2r+c)*256:+256][p, j] = W[r, j, k=c*128+p]; identity appended.
    wt = (weights.astype(np.float32).transpose(0, 2, 1)      # [r, k, j]
          .reshape(R, 2, P, BS).transpose(2, 0, 1, 3)        # [p, r, c, j]
          .reshape(P, WCOLS))
    return np.ascontiguousarray(
        np.concatenate([_round_fp22(wt), np.eye(P, dtype=np.float32)], axis=1))


def _in_maps(x: np.ndarray, weights: np.ndarray) -> list:
    xf = np.ascontiguousarray(x, dtype=np.float32).reshape(ROWS_TOTAL, D)
    wid = _prep_weights(np.asarray(weights))
    return [
        {"x": xf[i * ROWS_CORE:(i + 1) * ROWS_CORE], "w": wid}
        for i in range(N_CORES)
    ]


def kernel(x: np.ndarray, weights: np.ndarray) -> np.ndarray:
    nc = _get_nc()
    in_maps = _in_maps(x, weights)
    res = run_bass_kernel_spmd(nc, in_maps, list(range(N_CORES)))
    out = np.concatenate([res.results[i]["out"] for i in range(N_CORES)], axis=0)
    return out.reshape(x.shape).astype(np.float32, copy=False)


if __name__ == "__main__":
    xs = np.random.randn(4, 2048, D).astype(np.float32)
    ws = (np.broadcast_to(np.eye(BS, dtype=np.float32), (R, BS, BS))
          + 0.02 * np.random.randn(R, BS, BS).astype(np.float32))
    o = kernel(xs, ws)
    print("kernel ran, out shape", o.shape, o.dtype)



# revision 8
# speedup vs baseline: 1.5259x; 1.5259x over previous
"""Trainium2 Bass kernel for nn_BlockDiagonalLinear_text (hyperbolic block-diag linear).

Math: the reference's per-row operations reduce to
  out = alpha_row * y   with  y = x @ blockdiag(W_1..W_16).T
where alpha_row is a chain of tanh/artanh scalars of ||x_row|| and
||y_row|| (the expmap0 scale cancels; validated numerically at 1.6e-4).

Sharding: data-parallel over rows; 8192 rows -> 8 cores x 1024 rows,
weights replicated (bf16).

Per-core pipeline (8 tiles of 128 rows), all-bf16 datapath:
  SWDGE cast-DMA x (fp32 HBM -> bf16 SBUF) ->
  PE transpose x chunks (bf16, 1 cyc/row) -> DVE copy xt (bf16 2x mode) ->
  PE: per-chunk Gram matmul accumulates x@x^T (diag = ||x||^2, extracted
  with one DVE tensor_tensor_reduce against an identity mask) +
  block matmuls y = x @ W^T (bf16, fp32 PSUM) ->
  ACT copies y PSUM->SBUF (cast bf16) -> DVE tensor_tensor_reduce y*y
  for ||y||^2 -> per-row scalar chain batched over tile PAIRS ([128,2]
  ops; Ln/Exp only, single ACT table set preloaded once) ->
  DVE in-place scale (bf16 4x mode) -> SWDGE cast-DMA out (bf16 -> fp32).
"""
import sys
import numpy as np

for _p in ("/opt/trn_rl_repo", "/root/.axon_site/_ro/trn_rl_repo"):
    if _p not in sys.path:
        sys.path.append(_p)

import ml_dtypes
import concourse.bass as bass
import concourse.bacc as bacc
import concourse.mybir as mybir
from concourse import tile
from concourse.bass_utils import run_bass_kernel_spmd
from concourse.hw_specs import get_activation_tables

R, BS = 16, 256           # 16 diagonal blocks of 256x256
D = R * BS                # 4096
P = 128                   # partitions
N_CORES = 8
ROWS_TOTAL = 4 * 2048     # 8192
ROWS_CORE = ROWS_TOTAL // N_CORES   # 1024
NT = ROWS_CORE // P       # 8 tiles of 128 rows per core
NC = D // P               # 32 k-chunks of 128
WCOLS = 2 * R * BS        # 8192 weight columns
WIDC = WCOLS + P          # + bf16 identity columns

f32 = mybir.dt.float32
bf16 = mybir.dt.bfloat16
AF = mybir.ActivationFunctionType
OP = mybir.AluOpType

CLIP_Z = float(np.float32(1.0) - np.float32(1e-5))          # 0.99999
MAXNORM = float(np.float32(1.0 - 1e-3) / np.float32(0.1))   # 9.99


def build_nc():
    nc = bacc.Bacc()
    x_d = nc.declare_dram_parameter("x", [ROWS_CORE, D], f32, isOutput=False)
    w_d = nc.declare_dram_parameter("w", [P, WIDC], bf16, isOutput=False)
    m_d = nc.declare_dram_parameter("idm", [P, P], f32, isOutput=False)
    out_d = nc.declare_dram_parameter("out", [ROWS_CORE, D], f32, isOutput=True)

    tabs = list(get_activation_tables(nc.m.arch).items())
    nle_id = next(i for i, (n, _) in enumerate(tabs)
                  if n == "natural_log_exp_and_others")

    with tile.TileContext(nc) as tc:
        with (
            tc.tile_pool(name="wpool", bufs=1) as wpool,
            tc.tile_pool(name="xpool", bufs=3) as xpool,
            tc.tile_pool(name="xtpool", bufs=2) as xtpool,
            tc.tile_pool(name="ypool", bufs=4) as ypool,
            tc.tile_pool(name="sqpool", bufs=2) as sqpool,
            tc.tile_pool(name="stats", bufs=2) as stats,
            tc.tile_pool(name="pst", bufs=2, space="PSUM") as pst,
            tc.tile_pool(name="psy", bufs=2, space="PSUM") as psy,
            tc.tile_pool(name="psg", bufs=1, space="PSUM") as psg,
        ):
            V = nc.vector

            # ACT: preload the one table set with ln+exp+copy so the
            # auto-inserted per-function loads (which thrash between the
            # natural_log and exp_and_others sets) all become no-ops.
            nc.scalar.add_instruction(mybir.InstLoadActFuncSet(
                name=nc.get_next_instruction_name(),
                act_func_set_id=nle_id, ins=[], outs=[]))

            w_sb = wpool.tile([P, WIDC], bf16, name="w_sb")
            nc.sync.dma_start(out=w_sb[:], in_=w_d[:])
            id_sb = w_sb[:, WCOLS:WIDC]
            idm_sb = wpool.tile([P, P], f32, name="idm_sb")
            nc.sync.dma_start(out=idm_sb[:], in_=m_d[:])

            def st(shape, tag):
                return stats.tile(shape, f32, tag=tag, name=tag)

            qq = None
            for i in range(NT):
                t = i % 2
                xb = xpool.tile([P, D], bf16, tag="x", name=f"x_{i}")
                nc.gpsimd.dma_start(out=xb[:], in_=x_d[i * P:(i + 1) * P, :])

                if t == 0:
                    qq = st([P, 4], "qq")   # [qx_t0, qx_t1, qy_t0, qy_t1]

                # transpose x: 4 chunks of 128 per PSUM tile, then one
                # bf16 2x-mode DVE copy per group of 4
                xt = xtpool.tile([P, D], bf16, tag="xt", name=f"xt_{i}")
                gram = psg.tile([P, P], f32, tag="gram", name=f"gram_{i}")
                y_sb = ypool.tile([P, D], bf16, tag="y", name=f"y_{i}")
                for g in range(NC // 4):
                    tp = pst.tile([P, 4 * P], bf16, tag="tp", name=f"tp_{i}_{g}")
                    for c in range(4):
                        kc = 4 * g + c
                        nc.tensor.transpose(
                            tp[:, c * P:(c + 1) * P],
                            xb[:, kc * P:(kc + 1) * P], id_sb)
                    V.tensor_copy(xt[:, g * 4 * P:(g + 1) * 4 * P], tp[:])
                    # Gram: accumulate x @ x^T over all 32 chunks; its
                    # diagonal is the row-wise ||x||^2
                    for c in range(4):
                        kc = 4 * g + c
                        nc.tensor.matmul(
                            gram[:],
                            xt[:, kc * P:(kc + 1) * P],
                            xt[:, kc * P:(kc + 1) * P],
                            start=(kc == 0), stop=(kc == NC - 1),
                        )
                    # y block matmuls for the 2 blocks covered by this group
                    if g % 2 == 1:
                        py = psy.tile([P, 4 * BS], f32, tag="py",
                                      name=f"py_{i}_{g // 2}")
                        for rr in range(4):
                            r = 4 * (g // 2) + rr
                            for c in range(2):
                                kc = 2 * r + c
                                nc.tensor.matmul(
                                    py[:, rr * BS:(rr + 1) * BS],
                                    xt[:, kc * P:(kc + 1) * P],
                                    w_sb[:, kc * BS:(kc + 1) * BS],
                                    start=(c == 0), stop=(c == 1),
                                )
                        # drain 4 blocks at once: ACT copy PSUM -> SBUF bf16
                        nc.scalar.activation(
                            y_sb[:, (g // 2) * 4 * BS:(g // 2 + 1) * 4 * BS],
                            py[:], AF.Copy)

                # qx = diag(gram): mask with identity, then free-dim reduce
                gsc = sqpool.tile([P, P], f32, tag="gsc", name=f"gsc_{i}")
                V.tensor_mul(gsc[:], gram[:], idm_sb[:])
                V.reduce_sum(qq[:, t:t + 1], gsc[:], axis=mybir.AxisListType.X)
                # qy = sum y^2 on ACT (Square is in the preloaded table set)
                sq = sqpool.tile([P, D], bf16, tag="sq", name=f"sq_{i}")
                nc.scalar.activation(sq[:], y_sb[:], AF.Square,
                                     accum_out=qq[:, 2 + t:3 + t])

                if t == 0:
                    prev_y = y_sb
                    continue

                # ---- per-row scalar chain for the tile pair ([128,2]) ----
                qx = qq[:, 0:2]
                qy = qq[:, 2:4]
                lnq = st([P, 4], "lnq")
                nc.scalar.activation(lnq[:], qq[:], AF.Ln)
                U = st([P, 4], "U")     # [u | y_n] = sqrt via exp(0.5 ln q)
                nc.scalar.activation(U[:], lnq[:], AF.Exp, scale=0.5)

                t1 = st([P, 2], "t1")   # 0.1 * max(u, 1e-5)
                V.tensor_scalar(out=t1[:], in0=U[:, 0:2], scalar1=1e-5,
                                scalar2=0.1, op0=OP.max, op1=OP.mult)
                r1 = st([P, 2], "r1")
                V.reciprocal(r1[:], t1[:])
                args_ = st([P, 2], "args_")
                V.tensor_scalar_min(args_[:], t1[:], 15.0)
                Es = st([P, 2], "Es")
                nc.scalar.activation(Es[:], args_[:], AF.Exp, scale=2.0)
                e1 = st([P, 2], "e1")
                V.tensor_scalar_add(e1[:], Es[:], 1.0)
                r2 = st([P, 2], "r2")
                V.reciprocal(r2[:], e1[:])
                tsx = st([P, 2], "tsx")   # tanh(0.1 u_c)
                V.tensor_scalar(out=tsx[:], in0=r2[:], scalar1=-2.0,
                                scalar2=1.0, op0=OP.mult, op1=OP.add)
                za = st([P, 2], "za")
                V.tensor_scalar_min(za[:], tsx[:], CLIP_Z)
                L = st([P, 4], "L")
                V.tensor_scalar_add(L[:, 0:2], za[:], 1.0)
                V.tensor_scalar(out=L[:, 2:4], in0=za[:], scalar1=-1.0,
                                scalar2=1.0, op0=OP.mult, op1=OP.add)
                lnL = st([P, 4], "lnL")
                nc.scalar.activation(lnL[:], L[:], AF.Ln)
                d_ = st([P, 2], "d_")     # 2*artanh(za)
                V.tensor_sub(d_[:], lnL[:, 0:2], lnL[:, 2:4])
                yns = st([P, 2], "yns")
                V.tensor_scalar_max(yns[:], U[:, 2:4], 1e-20)
                w1 = st([P, 2], "w1")
                V.tensor_mul(w1[:], U[:, 2:4], r1[:])
                w2 = st([P, 2], "w2")
                V.tensor_mul(w2[:], w1[:], d_[:])
                argt = st([P, 2], "argt")
                V.tensor_scalar(out=argt[:], in0=w2[:], scalar1=0.05,
                                scalar2=15.0, op0=OP.mult, op1=OP.min)
                Et = st([P, 2], "Et")
                nc.scalar.activation(Et[:], argt[:], AF.Exp, scale=2.0)
                e2 = st([P, 2], "e2")
                V.tensor_scalar_add(e2[:], Et[:], 1.0)
                r3 = st([P, 2], "r3")
                V.reciprocal(r3[:], e2[:])
                ttx = st([P, 2], "ttx")   # tanh(arg_t)
                V.tensor_scalar(out=ttx[:], in0=r3[:], scalar1=-2.0,
                                scalar2=1.0, op0=OP.mult, op1=OP.add)
                nrm = st([P, 2], "nrm")
                V.tensor_scalar(out=nrm[:], in0=ttx[:], scalar1=10.0,
                                scalar2=1e-5, op0=OP.mult, op1=OP.max)
                ryn = st([P, 2], "ryn")
                V.reciprocal(ryn[:], yns[:])
                gs = st([P, 2], "gs")
                V.tensor_mul(gs[:], ttx[:], ryn[:])
                rn = st([P, 2], "rn")
                V.reciprocal(rn[:], nrm[:])
                pf = st([P, 2], "pf")
                V.tensor_scalar(out=pf[:], in0=rn[:], scalar1=MAXNORM,
                                scalar2=1.0, op0=OP.mult, op1=OP.min)
                m_ = st([P, 2], "m_")
                V.tensor_scalar_min(m_[:], nrm[:], MAXNORM)
                zb = st([P, 2], "zb")
                V.tensor_scalar_mul(zb[:], m_[:], 0.1)
                B = st([P, 4], "B")
                V.tensor_scalar_add(B[:, 0:2], zb[:], 1.0)
                V.tensor_scalar(out=B[:, 2:4], in0=zb[:], scalar1=-1.0,
                                scalar2=1.0, op0=OP.mult, op1=OP.add)
                lnB = st([P, 4], "lnB")
                nc.scalar.activation(lnB[:], B[:], AF.Ln)
                db = st([P, 2], "db")     # 2*artanh(0.1 m)
                V.tensor_sub(db[:], lnB[:, 0:2], lnB[:, 2:4])
                rzb = st([P, 2], "rzb")
                V.reciprocal(rzb[:], zb[:])
                a1 = st([P, 2], "a1")
                V.tensor_mul(a1[:], gs[:], pf[:])
                a2 = st([P, 2], "a2")
                V.tensor_mul(a2[:], db[:], rzb[:])
                al = st([P, 2], "al")
                V.tensor_mul(al[:], a1[:], a2[:])
                mask = st([P, 2], "mask")
                V.tensor_scalar(out=mask[:], in0=qy[:], scalar1=0.0,
                                scalar2=None, op0=OP.is_gt)
                alm = st([P, 2], "alm")
                V.tensor_mul(alm[:], al[:], mask[:])

                # scale both tiles of the pair in place (bf16 4x mode),
                # then cast-DMA out (bf16 -> fp32)
                for tt, yt in ((0, prev_y), (1, y_sb)):
                    ii = i - 1 + tt
                    V.tensor_scalar(out=yt[:], in0=yt[:],
                                    scalar1=alm[:, tt:tt + 1], scalar2=5.0,
                                    op0=OP.mult, op1=OP.mult)
                    nc.gpsimd.dma_start(out=out_d[ii * P:(ii + 1) * P, :],
                                        in_=yt[:])
    nc.finalize()
    return nc


_NC = None


def _get_nc():
    global _NC
    if _NC is None:
        _NC = build_nc()
    return _NC


def _prep_weights(weights: np.ndarray) -> np.ndarray:
    # w_sb[p, (2r+c)*256+j] = W[r, j, k=c*128+p]; bf16; identity appended.
    wt = (weights.astype(np.float32).transpose(0, 2, 1)      # [r, k, j]
          .reshape(R, 2, P, BS).transpose(2, 0, 1, 3)        # [p, r, c, j]
          .reshape(P, WCOLS))
    full = np.concatenate([wt, np.eye(P, dtype=np.float32)], axis=1)
    return np.ascontiguousarray(full).astype(ml_dtypes.bfloat16)


def _in_maps(x: np.ndarray, weights: np.ndarray) -> list:
    xf = np.ascontiguousarray(x, dtype=np.float32).reshape(ROWS_TOTAL, D)
    wid = _prep_weights(np.asarray(weights))
    idm = np.eye(P, dtype=np.float32)
    return [
        {"x": xf[i * ROWS_CORE:(i + 1) * ROWS_CORE], "w": wid, "idm": idm}
        for i in range(N_CORES)
    ]


def kernel(x: np.ndarray, weights: np.ndarray) -> np.ndarray:
    nc = _get_nc()
    in_maps = _in_maps(x, weights)
    res = run_bass_kernel_spmd(nc, in_maps, list(range(N_CORES)))
    out = np.concatenate([res.results[i]["out"] for i in range(N_CORES)], axis=0)
    return out.reshape(x.shape).astype(np.float32, copy=False)


if __name__ == "__main__":
    xs = np.random.randn(4, 2048, D).astype(np.float32)
    ws = (np.broadcast_to(np.eye(BS, dtype=np.float32), (R, BS, BS))
          + 0.02 * np.random.randn(R, BS, BS).astype(np.float32))
    o = kernel(xs, ws)
    print("kernel ran, out shape", o.shape, o.dtype)
